# revision 5
# baseline (speedup 1.0000x reference)
"""Masked graph-attention kernel for Trainium2, data-parallel over batch.

Problem: out = relu((softmax(mask*(QK^T) - NEG(1-mask)) @ V) @ Wo + bo)
         Q/K/V = relu(x @ W{q,k,v} + b{q,k,v}),  per independent graph.
Shapes:  x [128, 512, 256], mask [128, 512, 512], all weights [256,256].

Sharding: batch dim B=128 split across 8 NeuronCores (16 graphs each);
weights replicated; no collectives.

Layout strategy (v2): all transposes are done on the HOST during input
staging, so the device pipeline is transpose-free:
- x is shipped pre-transposed as xT [d, n] (bf16), so q^T/k^T (needed
  as matmul operands for scores) and v (natural) all come straight from
  matmuls against xT.
- mask is shipped pre-transposed as maskT [m, n] (bf16), so attention
  can be computed directly in transposed orientation: scoresT [m, n] =
  k^T-blocks^T @ q^T. exp(scoresT)*maskT is then immediately the
  right operand layout for the att@V matmul - no [512,512] transpose.
- The softmax denominator (a per-n partition-dim reduction in this
  orientation) is computed with K=128->1 ones-column matmuls, and the
  normalization is applied as a rank-1 broadcast (ones x recip row,
  one K=1 matmul) multiplied in during the PV PSUM->SBUF copy (DVE).
- exp(scores)*mask == the reference's masked softmax numerator exactly
  (mask is 0/1; the reference's -9e15 fill underflows exp to 0). No
  max-subtraction needed: scores < ~45 so exp stays in f32/bf16 range,
  and normalization happens after the PV matmul (linearity).
- Per-free-dim biases: bv via a scalar_tensor_tensor add against a
  host-shipped broadcast tile, bo via a K=1 ones x (bo|bo) PSUM seed.
  Per-partition biases bq/bk ride the ScalarE relu epilogue from a
  host-shipped [128,4] column tensor.

HBM traffic per graph: 256KB xT + 512KB maskT (bf16) + 512KB out (f32)
~= 1.25MB vs ~3MB for the f32-natural baseline; zero PE transposes.
"""

import numpy as np

B, N, DIN, H, DOUT = 128, 512, 256, 256, 256
N_CORES = 8
GPC = B // N_CORES  # graphs per core

P = 128          # partitions
NT = N // P      # 4 row/col tiles per graph
DT = DIN // P    # 2 contraction tiles for x
HT = H // P      # 2 hidden tiles

_compiled = {}


def build(n_graphs=GPC):
    import concourse.bass as bass
    import concourse.mybir as mybir
    import concourse.tile as tile
    from concourse import bacc

    f32 = mybir.dt.float32
    bf16 = mybir.dt.bfloat16
    Relu = mybir.ActivationFunctionType.Relu
    Exp = mybir.ActivationFunctionType.Exp
    ADD = mybir.AluOpType.add
    MULT = mybir.AluOpType.mult

    nc = bacc.Bacc("TRN2")
    xt_d = nc.dram_tensor("xt", [n_graphs, P, DT, N], bf16, kind="ExternalInput")
    mt_d = nc.dram_tensor("mt", [n_graphs, P, NT, N], bf16, kind="ExternalInput")
    wq_d = nc.dram_tensor("wq", [P, DT, H], bf16, kind="ExternalInput")
    wk_d = nc.dram_tensor("wk", [P, DT, H], bf16, kind="ExternalInput")
    wv_d = nc.dram_tensor("wv", [P, DT, H], bf16, kind="ExternalInput")
    wo_d = nc.dram_tensor("wo", [P, HT, DOUT], bf16, kind="ExternalInput")
    bqk_d = nc.dram_tensor("bqk", [P, 2 * HT], f32, kind="ExternalInput")
    bvb_d = nc.dram_tensor("bvb", [P, 2, H], f32, kind="ExternalInput")
    bo2_d = nc.dram_tensor("bo2", [1, 2, DOUT], bf16, kind="ExternalInput")
    out_d = nc.dram_tensor("out", [n_graphs, P, NT, DOUT], f32, kind="ExternalOutput")

    with tile.TileContext(nc) as tc:
        with (
            tc.tile_pool(name="singles", bufs=1) as singles,
            tc.tile_pool(name="xin", bufs=3) as xin_pool,
            tc.tile_pool(name="min", bufs=3) as min_pool,
            tc.tile_pool(name="qk", bufs=2) as qk_pool,
            tc.tile_pool(name="vp", bufs=2) as v_pool,
            tc.tile_pool(name="ep", bufs=2) as e_pool,
            tc.tile_pool(name="o1", bufs=2) as o1_pool,
            tc.tile_pool(name="tmp", bufs=3) as tmp_pool,
            tc.tile_pool(name="small", bufs=8) as small,
            tc.tile_pool(name="outp", bufs=3) as outp,
            tc.tile_pool(name="psa", bufs=4, space="PSUM") as psa,
            tc.tile_pool(name="pspv", bufs=1, space="PSUM") as pspv,
            tc.tile_pool(name="psden", bufs=1, space="PSUM") as psden,
            tc.tile_pool(name="psrb", bufs=1, space="PSUM") as psrb,
        ):
            # ---- one-time constants (all host-shipped, just DMA in) ----
            w_sb = {}
            for nm, d in (("wq", wq_d), ("wk", wk_d), ("wv", wv_d), ("wo", wo_d)):
                t = singles.tile([P, DT, 256], bf16, tag=f"w_{nm}")
                nc.sync.dma_start(out=t, in_=d[:])
                w_sb[nm] = t
            bqk = singles.tile([P, 2 * HT], f32, tag="bqk")
            nc.sync.dma_start(out=bqk, in_=bqk_d[:])
            bvb = singles.tile([P, 2, H], f32, tag="bvb")
            nc.sync.dma_start(out=bvb, in_=bvb_d[:])
            bo2 = singles.tile([1, 2, DOUT], bf16, tag="bo2")
            nc.sync.dma_start(out=bo2, in_=bo2_d[:])
            ones_col = singles.tile([P, 1], bf16, tag="ones_col")
            nc.vector.memset(ones_col, 1.0)
            ones_row = singles.tile([1, P], bf16, tag="ones_row")
            nc.vector.memset(ones_row, 1.0)

            def part1(g):
                """loads, qT/kT/v, scoresT, exp*mask, den, PV psums."""
                xT = xin_pool.tile([P, DT, N], bf16, tag="xT")
                nc.sync.dma_start(out=xT, in_=xt_d[g])
                mT = min_pool.tile([P, NT, N], bf16, tag="mT")
                nc.sync.dma_start(out=mT, in_=mt_d[g])

                # qT, kT [h-part, n] = relu(W^T xT + b); bias per-partition
                qT = qk_pool.tile([P, HT, N], bf16, tag="qT")
                kT = qk_pool.tile([P, HT, N], bf16, tag="kT")
                for wi, (wnm, dstT) in enumerate((("wq", qT), ("wk", kT))):
                    for hh in range(HT):
                        ps = psa.tile([P, N], f32, tag="b512")
                        for dd in range(DT):
                            nc.tensor.matmul(
                                ps,
                                w_sb[wnm][:, dd, hh * P : (hh + 1) * P],
                                xT[:, dd, :],
                                start=(dd == 0),
                                stop=(dd == DT - 1),
                            )
                        nc.scalar.activation(
                            dstT[:, hh, :],
                            ps,
                            Relu,
                            bias=bqk[:, wi * HT + hh : wi * HT + hh + 1],
                            scale=1.0,
                        )

                # v natural [m-part, h]; bias+relu via DVE (2 passes)
                v_sb = v_pool.tile([P, NT, H], bf16, tag="v")
                for ip in range(NT // 2):
                    ps = psa.tile([P, N], f32, tag="b512")
                    for t2 in range(2):
                        j = 2 * ip + t2
                        for dd in range(DT):
                            nc.tensor.matmul(
                                ps[:, t2 * H : (t2 + 1) * H],
                                xT[:, dd, j * P : (j + 1) * P],
                                w_sb["wv"][:, dd, :],
                                start=(dd == 0),
                                stop=(dd == DT - 1),
                            )
                    vtmp = tmp_pool.tile([P, 2, H], bf16, tag="vtmp")
                    nc.vector.scalar_tensor_tensor(
                        out=vtmp,
                        in0=ps.rearrange("p (t h) -> p t h", t=2),
                        scalar=1.0,
                        in1=bvb,
                        op0=MULT,
                        op1=ADD,
                    )
                    nc.vector.tensor_scalar_max(
                        v_sb[:, 2 * ip : 2 * ip + 2, :], vtmp, 0.0
                    )

                # scoresT -> exp -> *maskT  (m on partitions, n free)
                eT = e_pool.tile([P, NT, N], bf16, tag="eT")
                for j in range(NT):
                    ps = psa.tile([P, N], f32, tag="b512")
                    for hh in range(HT):
                        nc.tensor.matmul(
                            ps,
                            kT[:, hh, j * P : (j + 1) * P],
                            qT[:, hh, :],
                            start=(hh == 0),
                            stop=(hh == HT - 1),
                        )
                    nc.scalar.activation(eT[:, j, :], ps, Exp)
                    nc.vector.tensor_mul(eT[:, j, :], eT[:, j, :], mT[:, j, :])

                # den[n] = sum_m eT (ones-column matmuls), interleaved with
                # the PV matmuls so the PE never waits on the DVE mask pass
                ps_den = psden.tile([1, N], f32, tag="den")
                ps_pv0 = pspv.tile([P, N], f32, tag="pv0")
                ps_pv1 = pspv.tile([P, N], f32, tag="pv1")
                ps_pv = [ps_pv0, ps_pv1]
                for j in range(NT - 1):
                    nc.tensor.matmul(
                        ps_den, ones_col, eT[:, j, :],
                        start=(j == 0), stop=False,
                    )
                for j in range(NT - 1):
                    nc.tensor.matmul(
                        ps_pv[0],
                        v_sb[:, j, 0:P],
                        eT[:, j, :],
                        start=(j == 0),
                        stop=False,
                    )
                nc.tensor.matmul(
                    ps_den, ones_col, eT[:, NT - 1, :], start=False, stop=True
                )
                nc.tensor.matmul(
                    ps_pv[0], v_sb[:, NT - 1, 0:P], eT[:, NT - 1, :],
                    start=False, stop=True,
                )
                recip = small.tile([1, N], f32, tag="recip")
                nc.vector.reciprocal(recip, ps_den)
                recip_bf = small.tile([1, N], bf16, tag="recip_bf")
                nc.vector.tensor_copy(recip_bf, recip)
                for j in range(NT):
                    nc.tensor.matmul(
                        ps_pv[1],
                        v_sb[:, j, P : 2 * P],
                        eT[:, j, :],
                        start=(j == 0),
                        stop=(j == NT - 1),
                    )
                return ps_pv, recip_bf

            def part1b(g, ps_pv, recip_bf):
                """rank-1 recip broadcast; normalize O1T during psum copy."""
                ps_rb = psrb.tile([P, N], f32, tag="rb")
                nc.tensor.matmul(ps_rb, ones_row, recip_bf, start=True, stop=True)
                rb_sb = tmp_pool.tile([P, N], f32, tag="rb_sb")
                nc.vector.tensor_copy(rb_sb, ps_rb)
                O1T = o1_pool.tile([P, HT, N], bf16, tag="O1T")
                for hh in range(HT):
                    nc.vector.tensor_mul(O1T[:, hh, :], ps_pv[hh], rb_sb)
                return O1T

            def part2(g, O1T):
                """out = relu(O1T^T-blocks @ Wo + bo), store."""
                outf = outp.tile([P, NT, DOUT], f32, tag="outf")
                for ip in range(NT // 2):
                    ps = psa.tile([P, N], f32, tag="b512")
                    nc.tensor.matmul(
                        ps.rearrange("p (t h) -> p t h", t=2),
                        ones_row,
                        bo2,
                        start=True,
                        stop=False,
                    )
                    for t2 in range(2):
                        i = 2 * ip + t2
                        for hh in range(HT):
                            nc.tensor.matmul(
                                ps[:, t2 * DOUT : (t2 + 1) * DOUT],
                                O1T[:, hh, i * P : (i + 1) * P],
                                w_sb["wo"][:, hh, :],
                                start=False,
                                stop=(t2 == 1 and hh == HT - 1),
                            )
                    nc.scalar.activation(outf[:, 2 * ip : 2 * ip + 2, :], ps, Relu)
                nc.gpsimd.dma_start(out=out_d[g], in_=outf)

            # software pipeline: out-projection of graph g-1 is emitted
            # between PV(g) and the recip-broadcast of g, so the PE never
            # waits on the DVE recip chain and O1T-normalize of g
            prev = None
            for g in range(n_graphs):
                st = part1(g)
                if prev is not None:
                    part2(*prev)
                prev = (g, part1b(g, *st))
            part2(*prev)

    nc.compile()
    return nc


def _get_compiled(n_graphs=GPC):
    if n_graphs not in _compiled:
        _compiled[n_graphs] = build(n_graphs)
    return _compiled[n_graphs]


def _in_maps(inputs):
    import ml_dtypes

    bf16 = ml_dtypes.bfloat16
    f32 = np.float32
    Wq = inputs["Wq"]
    Wk = inputs["Wk"]
    Wv = inputs["Wv"]
    Wo = inputs["Wo"]
    bq = np.asarray(inputs["bq"], f32)
    bk = np.asarray(inputs["bk"], f32)
    bv = np.asarray(inputs["bv"], f32)
    bo = np.asarray(inputs["bo"], f32)

    def wT(W):  # [256, 256] -> [p, dd, h] with d = 128*dd + p
        return np.ascontiguousarray(
            np.asarray(W, f32).reshape(DT, P, 256).transpose(1, 0, 2)
        ).astype(bf16)

    shared = {
        "wq": wT(Wq),
        "wk": wT(Wk),
        "wv": wT(Wv),
        "wo": wT(Wo),
        "bqk": np.ascontiguousarray(
            np.stack([bq[0:P], bq[P : 2 * P], bk[0:P], bk[P : 2 * P]], axis=1)
        ),
        "bvb": np.ascontiguousarray(np.broadcast_to(bv, (P, 2, H))),
        "bo2": np.ascontiguousarray(np.broadcast_to(bo, (1, 2, DOUT))).astype(bf16),
    }
    x = np.asarray(inputs["x"], f32)
    mask = np.asarray(inputs["mask"], f32)
    in_maps = []
    for c in range(N_CORES):
        sl = slice(c * GPC, (c + 1) * GPC)
        m = dict(shared)
        # xT [g, p, dd, n] = x[g, n, 128*dd + p]
        m["xt"] = np.ascontiguousarray(
            x[sl].transpose(0, 2, 1).reshape(GPC, DT, P, N).transpose(0, 2, 1, 3)
        ).astype(bf16)
        # maskT [g, p, j, n] = mask[g, n, 128*j + p]
        m["mt"] = np.ascontiguousarray(
            mask[sl].transpose(0, 2, 1).reshape(GPC, NT, P, N).transpose(0, 2, 1, 3)
        ).astype(bf16)
        in_maps.append(m)
    return in_maps


def _unshard_out(results):
    # out [g, p, t, o] -> [g, 128*t + p, o]
    outs = []
    for r in results:
        o = r["out"]
        outs.append(o.transpose(0, 2, 1, 3).reshape(GPC, N, DOUT))
    return np.concatenate(outs, axis=0)


def run(inputs, **kw):
    """Run on 8 NeuronCores; returns (out [B,N,DOUT], results list)."""
    from concourse.bass2jax import run_bass_via_pjrt

    nc = _get_compiled()
    results = run_bass_via_pjrt(nc, _in_maps(inputs), n_cores=N_CORES)
    out = _unshard_out(results)
    return out, results


def kernel(**inputs):
    out, _ = run(inputs)
    return out


def bench(inputs, iters=30, nc=None):
    """Run + time the jitted 8-core executable on device-resident buffers.

    Returns (out [B,N,DOUT], timing dict). Timing excludes host<->device
    transfer: inputs are staged once, then the same call is issued
    `iters` times; `pipelined_ns` is total/iters with async dispatch
    (overlapped RPC overhead), `blocked_ns` is the min per-call
    block_until_ready wall time (includes one dispatch round-trip).
    """
    import time

    import jax
    import concourse.mybir as mybir
    from concourse.bass2jax import (
        _bass_exec_p,
        install_neuronx_cc_hook,
        partition_id_tensor,
    )
    from jax.experimental.shard_map import shard_map
    from jax.sharding import Mesh, PartitionSpec

    install_neuronx_cc_hook()
    if nc is None:
        nc = _get_compiled()
    in_maps = _in_maps(inputs)

    partition_name = nc.partition_id_tensor.name if nc.partition_id_tensor else None
    in_names, out_names, out_avals, zero_outs = [], [], [], []
    for alloc in nc.m.functions[0].allocations:
        if not isinstance(alloc, mybir.MemoryLocationSet):
            continue
        name = alloc.memorylocations[0].name
        if alloc.kind == "ExternalInput":
            if name != partition_name:
                in_names.append(name)
        elif alloc.kind == "ExternalOutput":
            out_names.append(name)
            np_dt = mybir.dt.np(alloc.dtype)
            out_avals.append(
                jax.core.ShapedArray(tuple(alloc.tensor_shape), np_dt)
            )
            zero_outs.append(np.zeros(tuple(alloc.tensor_shape), np_dt))
    n_params = len(in_names)
    all_in_names = in_names + out_names
    if partition_name is not None:
        all_in_names = all_in_names + [partition_name]

    def _body(*args):
        operands = list(args)
        if partition_name is not None:
            operands.append(partition_id_tensor())
        outs = _bass_exec_p.bind(
            *operands,
            out_avals=tuple(out_avals),
            in_names=tuple(all_in_names),
            out_names=tuple(out_names),
            lowering_input_output_aliases=(),
            sim_require_finite=True,
            sim_require_nnan=True,
            nc=nc,
        )
        return tuple(outs)

    devices = jax.devices()[:N_CORES]
    mesh = Mesh(np.asarray(devices), ("core",))
    nin = n_params + len(out_names)
    sharded = jax.jit(
        shard_map(
            _body,
            mesh=mesh,
            in_specs=(PartitionSpec("core"),) * nin,
            out_specs=(PartitionSpec("core"),) * len(out_names),
            check_rep=False,
        ),
        keep_unused=True,
    )
    concat_in = [
        np.concatenate([np.asarray(in_maps[c][nm]) for c in range(N_CORES)], axis=0)
        for nm in in_names
    ]
    concat_zero = [
        np.zeros((N_CORES * z.shape[0], *z.shape[1:]), z.dtype) for z in zero_outs
    ]
    sharding = jax.sharding.NamedSharding(mesh, PartitionSpec("core"))
    dev_in = [jax.device_put(a, sharding) for a in concat_in + concat_zero]

    # warmup (compile + first exec); snapshot the output before any
    # further executions can recycle buffers
    t0 = time.time()
    out_arrs = sharded(*dev_in)
    jax.block_until_ready(out_arrs)
    out_np = np.asarray(out_arrs[0]).copy()
    warm_s = time.time() - t0

    blocked = []
    for _ in range(5):
        t0 = time.perf_counter()
        r = sharded(*dev_in)
        jax.block_until_ready(r)
        blocked.append(time.perf_counter() - t0)

    t0 = time.perf_counter()
    r = None
    for _ in range(iters):
        r = sharded(*dev_in)
    jax.block_until_ready(r)
    pipelined = (time.perf_counter() - t0) / iters

    out = _unshard_out(
        [{"out": out_np[c * GPC : (c + 1) * GPC]} for c in range(N_CORES)]
    )
    timing = {
        "warmup_s": warm_s,
        "blocked_ns": min(blocked) * 1e9,
        "pipelined_ns": pipelined * 1e9,
    }
    return out, timing


# revision 9
# speedup vs baseline: 1.2972x; 1.2972x over previous
"""Masked graph-attention kernel for Trainium2, data-parallel over batch.

Problem: out = relu((softmax(mask*(QK^T) - NEG(1-mask)) @ V) @ Wo + bo)
         Q/K/V = relu(x @ W{q,k,v} + b{q,k,v}),  per independent graph.
Shapes:  x [128, 512, 256], mask [128, 512, 512], all weights [256,256].

Sharding: batch dim B=128 split across 8 NeuronCores (16 graphs each);
weights replicated; no collectives.

Layout strategy (v2): all transposes are done on the HOST during input
staging, so the device pipeline is transpose-free:
- x is shipped pre-transposed as xT [d, n] (bf16), so q^T/k^T (needed
  as matmul operands for scores) and v (natural) all come straight from
  matmuls against xT.
- mask is shipped pre-transposed as maskT [m, n] (bf16), so attention
  can be computed directly in transposed orientation: scoresT [m, n] =
  k^T-blocks^T @ q^T. exp(scoresT)*maskT is then immediately the
  right operand layout for the att@V matmul - no [512,512] transpose.
- The softmax denominator (a per-n partition-dim reduction in this
  orientation) is computed with K=128->1 ones-column matmuls, and the
  normalization is applied as a rank-1 broadcast (ones x recip row,
  one K=1 matmul) multiplied in during the PV PSUM->SBUF copy (DVE).
- exp(scores)*mask == the reference's masked softmax numerator exactly
  (mask is 0/1; the reference's -9e15 fill underflows exp to 0). No
  max-subtraction needed: scores < ~45 so exp stays in f32/bf16 range,
  and normalization happens after the PV matmul (linearity).
- Per-free-dim biases: bv via a scalar_tensor_tensor add against a
  host-shipped broadcast tile, bo via a K=1 ones x (bo|bo) PSUM seed.
  Per-partition biases bq/bk ride the ScalarE relu epilogue from a
  host-shipped [128,4] column tensor.

HBM traffic per graph: 256KB xT + 512KB maskT (bf16) + 512KB out (f32)
~= 1.25MB vs ~3MB for the f32-natural baseline; zero PE transposes.
"""

import numpy as np

B, N, DIN, H, DOUT = 128, 512, 256, 256, 256
N_CORES = 8
GPC = B // N_CORES  # graphs per core

P = 128          # partitions
NT = N // P      # 4 row/col tiles per graph
DT = DIN // P    # 2 contraction tiles for x
HT = H // P      # 2 hidden tiles

_compiled = {}


def build(n_graphs=GPC):
    import concourse.bass as bass
    import concourse.mybir as mybir
    import concourse.tile as tile
    from concourse import bacc

    f32 = mybir.dt.float32
    bf16 = mybir.dt.bfloat16
    Relu = mybir.ActivationFunctionType.Relu
    Exp = mybir.ActivationFunctionType.Exp
    ADD = mybir.AluOpType.add
    MULT = mybir.AluOpType.mult

    nc = bacc.Bacc("TRN2")
    xt_d = nc.dram_tensor("xt", [n_graphs, P, DT, N], bf16, kind="ExternalInput")
    mt_d = nc.dram_tensor("mt", [n_graphs, P, NT, N], bf16, kind="ExternalInput")
    wq_d = nc.dram_tensor("wq", [P, DT, H], bf16, kind="ExternalInput")
    wk_d = nc.dram_tensor("wk", [P, DT, H], bf16, kind="ExternalInput")
    wv_d = nc.dram_tensor("wv", [P, DT, H], bf16, kind="ExternalInput")
    wo_d = nc.dram_tensor("wo", [P, HT, DOUT], bf16, kind="ExternalInput")
    bqk_d = nc.dram_tensor("bqk", [P, 2 * HT], f32, kind="ExternalInput")
    bvb_d = nc.dram_tensor("bvb", [P, 2, H], f32, kind="ExternalInput")
    bo2_d = nc.dram_tensor("bo2", [1, 2, DOUT], bf16, kind="ExternalInput")
    out_d = nc.dram_tensor("out", [n_graphs, P, NT, DOUT], f32, kind="ExternalOutput")

    with tile.TileContext(nc) as tc:
        with (
            tc.tile_pool(name="singles", bufs=1) as singles,
            tc.tile_pool(name="xin", bufs=3) as xin_pool,
            tc.tile_pool(name="min", bufs=3) as min_pool,
            tc.tile_pool(name="qk", bufs=2) as qk_pool,
            tc.tile_pool(name="vp", bufs=2) as v_pool,
            tc.tile_pool(name="ep", bufs=2) as e_pool,
            tc.tile_pool(name="o1", bufs=2) as o1_pool,
            tc.tile_pool(name="tmp", bufs=3) as tmp_pool,
            tc.tile_pool(name="small", bufs=8) as small,
            tc.tile_pool(name="outp", bufs=3) as outp,
            tc.tile_pool(name="psa", bufs=3, space="PSUM") as psa,
            tc.tile_pool(name="psout", bufs=2, space="PSUM") as psout,
            tc.tile_pool(name="pspv", bufs=1, space="PSUM") as pspv,
            tc.tile_pool(name="psden", bufs=1, space="PSUM") as psden,
        ):
            # ---- one-time constants (all host-shipped, just DMA in) ----
            w_sb = {}
            for nm, d in (("wq", wq_d), ("wk", wk_d), ("wv", wv_d), ("wo", wo_d)):
                t = singles.tile([P, DT, 256], bf16, tag=f"w_{nm}")
                nc.sync.dma_start(out=t, in_=d[:])
                w_sb[nm] = t
            bqk = singles.tile([P, 2 * HT], f32, tag="bqk")
            nc.sync.dma_start(out=bqk, in_=bqk_d[:])
            bvb = singles.tile([P, 2, H], f32, tag="bvb")
            nc.sync.dma_start(out=bvb, in_=bvb_d[:])
            bo2 = singles.tile([1, 2, DOUT], bf16, tag="bo2")
            nc.sync.dma_start(out=bo2, in_=bo2_d[:])
            ones_col = singles.tile([P, 1], bf16, tag="ones_col")
            nc.vector.memset(ones_col, 1.0)
            ones_row = singles.tile([1, P], bf16, tag="ones_row")
            nc.vector.memset(ones_row, 1.0)

            def part1(g):
                """loads, qT/kT/v, scoresT, exp*mask, den, PV psums."""
                xT = xin_pool.tile([P, DT, N], bf16, tag="xT")
                nc.sync.dma_start(out=xT, in_=xt_d[g])
                mT = min_pool.tile([P, NT, N], bf16, tag="mT")
                nc.sync.dma_start(out=mT, in_=mt_d[g])

                # qT, kT [h-part, n] = relu(W^T xT + b); bias per-partition
                qT = qk_pool.tile([P, HT, N], bf16, tag="qT")
                kT = qk_pool.tile([P, HT, N], bf16, tag="kT")
                for wi, (wnm, dstT) in enumerate((("wq", qT), ("wk", kT))):
                    for hh in range(HT):
                        ps = psa.tile([P, N], f32, tag="b512")
                        for dd in range(DT):
                            nc.tensor.matmul(
                                ps,
                                w_sb[wnm][:, dd, hh * P : (hh + 1) * P],
                                xT[:, dd, :],
                                start=(dd == 0),
                                stop=(dd == DT - 1),
                            )
                        nc.scalar.activation(
                            dstT[:, hh, :],
                            ps,
                            Relu,
                            bias=bqk[:, wi * HT + hh : wi * HT + hh + 1],
                            scale=1.0,
                        )

                # v natural [m-part, h]; bias+relu via DVE (2 passes)
                v_sb = v_pool.tile([P, NT, H], bf16, tag="v")
                for ip in range(NT // 2):
                    ps = psa.tile([P, N], f32, tag="b512")
                    for t2 in range(2):
                        j = 2 * ip + t2
                        for dd in range(DT):
                            nc.tensor.matmul(
                                ps[:, t2 * H : (t2 + 1) * H],
                                xT[:, dd, j * P : (j + 1) * P],
                                w_sb["wv"][:, dd, :],
                                start=(dd == 0),
                                stop=(dd == DT - 1),
                            )
                    vtmp = tmp_pool.tile([P, 2, H], bf16, tag="vtmp")
                    nc.vector.scalar_tensor_tensor(
                        out=vtmp,
                        in0=ps.rearrange("p (t h) -> p t h", t=2),
                        scalar=1.0,
                        in1=bvb,
                        op0=MULT,
                        op1=ADD,
                    )
                    nc.vector.tensor_scalar_max(
                        v_sb[:, 2 * ip : 2 * ip + 2, :], vtmp, 0.0
                    )

                # scoresT -> exp -> *maskT  (m on partitions, n free)
                eT = e_pool.tile([P, NT, N], bf16, tag="eT")
                for j in range(NT):
                    ps = psa.tile([P, N], f32, tag="b512")
                    for hh in range(HT):
                        nc.tensor.matmul(
                            ps,
                            kT[:, hh, j * P : (j + 1) * P],
                            qT[:, hh, :],
                            start=(hh == 0),
                            stop=(hh == HT - 1),
                        )
                    nc.scalar.activation(eT[:, j, :], ps, Exp)
                    nc.vector.tensor_mul(eT[:, j, :], eT[:, j, :], mT[:, j, :])

                # den[n] = sum_m eT (ones-column matmuls), interleaved with
                # the PV matmuls so the PE never waits on the DVE mask pass
                ps_den = psden.tile([1, N], f32, tag="den")
                ps_pv0 = pspv.tile([P, N], f32, tag="pv0")
                ps_pv1 = pspv.tile([P, N], f32, tag="pv1")
                ps_pv = [ps_pv0, ps_pv1]
                for j in range(NT - 1):
                    nc.tensor.matmul(
                        ps_den, ones_col, eT[:, j, :],
                        start=(j == 0), stop=False,
                    )
                for j in range(NT - 1):
                    nc.tensor.matmul(
                        ps_pv[0],
                        v_sb[:, j, 0:P],
                        eT[:, j, :],
                        start=(j == 0),
                        stop=False,
                    )
                nc.tensor.matmul(
                    ps_den, ones_col, eT[:, NT - 1, :], start=False, stop=True
                )
                nc.tensor.matmul(
                    ps_pv[0], v_sb[:, NT - 1, 0:P], eT[:, NT - 1, :],
                    start=False, stop=True,
                )
                recip = small.tile([1, N], f32, tag="recip")
                # approx (51-ULP) reciprocal: the exact one is ~6 cyc/elem
                # and, single-partition, would sit 3.3us on the DVE queue
                nc.vector.reciprocal_approx_fast(recip, ps_den)
                recip_bf = small.tile([1, N], bf16, tag="recip_bf")
                nc.vector.tensor_copy(recip_bf, recip)
                for j in range(NT):
                    nc.tensor.matmul(
                        ps_pv[1],
                        v_sb[:, j, P : 2 * P],
                        eT[:, j, :],
                        start=(j == 0),
                        stop=(j == NT - 1),
                    )
                return ps_pv, recip_bf

            def part1b(g, ps_pv, recip_bf):
                """rank-1 recip broadcast; normalize O1T during psum copy."""
                ps_rb = psa.tile([P, N], f32, tag="b512")
                nc.tensor.matmul(ps_rb, ones_row, recip_bf, start=True, stop=True)
                rb_sb = tmp_pool.tile([P, N], f32, tag="rb_sb")
                nc.vector.tensor_copy(rb_sb, ps_rb)
                O1T = o1_pool.tile([P, HT, N], bf16, tag="O1T")
                for hh in range(HT):
                    nc.vector.tensor_mul(O1T[:, hh, :], ps_pv[hh], rb_sb)
                return O1T

            def part2(g, O1T):
                """out = relu(O1T^T-blocks @ Wo + bo), store."""
                outf = outp.tile([P, NT, DOUT], f32, tag="outf")
                for ip in range(NT // 2):
                    ps = psout.tile([P, N], f32, tag="o512")
                    nc.tensor.matmul(
                        ps.rearrange("p (t h) -> p t h", t=2),
                        ones_row,
                        bo2,
                        start=True,
                        stop=False,
                    )
                    for t2 in range(2):
                        i = 2 * ip + t2
                        for hh in range(HT):
                            nc.tensor.matmul(
                                ps[:, t2 * DOUT : (t2 + 1) * DOUT],
                                O1T[:, hh, i * P : (i + 1) * P],
                                w_sb["wo"][:, hh, :],
                                start=False,
                                stop=(t2 == 1 and hh == HT - 1),
                            )
                    nc.scalar.activation(outf[:, 2 * ip : 2 * ip + 2, :], ps, Relu)
                nc.gpsimd.dma_start(out=out_d[g], in_=outf)

            # software pipeline: out-projection of graph g-1 is emitted
            # between PV(g) and the recip-broadcast of g, so the PE never
            # waits on the DVE recip chain and O1T-normalize of g
            prev = None
            for g in range(n_graphs):
                st = part1(g)
                if prev is not None:
                    part2(*prev)
                prev = (g, part1b(g, *st))
            part2(*prev)

    nc.compile()
    return nc


def _get_compiled(n_graphs=GPC):
    if n_graphs not in _compiled:
        _compiled[n_graphs] = build(n_graphs)
    return _compiled[n_graphs]


def _in_maps(inputs):
    import ml_dtypes

    bf16 = ml_dtypes.bfloat16
    f32 = np.float32
    Wq = inputs["Wq"]
    Wk = inputs["Wk"]
    Wv = inputs["Wv"]
    Wo = inputs["Wo"]
    bq = np.asarray(inputs["bq"], f32)
    bk = np.asarray(inputs["bk"], f32)
    bv = np.asarray(inputs["bv"], f32)
    bo = np.asarray(inputs["bo"], f32)

    def wT(W):  # [256, 256] -> [p, dd, h] with d = 128*dd + p
        return np.ascontiguousarray(
            np.asarray(W, f32).reshape(DT, P, 256).transpose(1, 0, 2)
        ).astype(bf16)

    shared = {
        "wq": wT(Wq),
        "wk": wT(Wk),
        "wv": wT(Wv),
        "wo": wT(Wo),
        "bqk": np.ascontiguousarray(
            np.stack([bq[0:P], bq[P : 2 * P], bk[0:P], bk[P : 2 * P]], axis=1)
        ),
        "bvb": np.ascontiguousarray(np.broadcast_to(bv, (P, 2, H))),
        "bo2": np.ascontiguousarray(np.broadcast_to(bo, (1, 2, DOUT))).astype(bf16),
    }
    x = np.asarray(inputs["x"], f32)
    mask = np.asarray(inputs["mask"], f32)
    in_maps = []
    for c in range(N_CORES):
        sl = slice(c * GPC, (c + 1) * GPC)
        m = dict(shared)
        # xT [g, p, dd, n] = x[g, n, 128*dd + p]
        m["xt"] = np.ascontiguousarray(
            x[sl].transpose(0, 2, 1).reshape(GPC, DT, P, N).transpose(0, 2, 1, 3)
        ).astype(bf16)
        # maskT [g, p, j, n] = mask[g, n, 128*j + p]
        m["mt"] = np.ascontiguousarray(
            mask[sl].transpose(0, 2, 1).reshape(GPC, NT, P, N).transpose(0, 2, 1, 3)
        ).astype(bf16)
        in_maps.append(m)
    return in_maps


def _unshard_out(results):
    # out [g, p, t, o] -> [g, 128*t + p, o]
    outs = []
    for r in results:
        o = r["out"]
        outs.append(o.transpose(0, 2, 1, 3).reshape(GPC, N, DOUT))
    return np.concatenate(outs, axis=0)


def run(inputs, **kw):
    """Run on 8 NeuronCores; returns (out [B,N,DOUT], results list)."""
    from concourse.bass2jax import run_bass_via_pjrt

    nc = _get_compiled()
    results = run_bass_via_pjrt(nc, _in_maps(inputs), n_cores=N_CORES)
    out = _unshard_out(results)
    return out, results


def kernel(**inputs):
    out, _ = run(inputs)
    return out


def bench(inputs, iters=30, nc=None):
    """Run + time the jitted 8-core executable on device-resident buffers.

    Returns (out [B,N,DOUT], timing dict). Timing excludes host<->device
    transfer: inputs are staged once, then the same call is issued
    `iters` times; `pipelined_ns` is total/iters with async dispatch
    (overlapped RPC overhead), `blocked_ns` is the min per-call
    block_until_ready wall time (includes one dispatch round-trip).
    """
    import time

    import jax
    import concourse.mybir as mybir
    from concourse.bass2jax import (
        _bass_exec_p,
        install_neuronx_cc_hook,
        partition_id_tensor,
    )
    from jax.experimental.shard_map import shard_map
    from jax.sharding import Mesh, PartitionSpec

    install_neuronx_cc_hook()
    if nc is None:
        nc = _get_compiled()
    in_maps = _in_maps(inputs)

    partition_name = nc.partition_id_tensor.name if nc.partition_id_tensor else None
    in_names, out_names, out_avals, zero_outs = [], [], [], []
    for alloc in nc.m.functions[0].allocations:
        if not isinstance(alloc, mybir.MemoryLocationSet):
            continue
        name = alloc.memorylocations[0].name
        if alloc.kind == "ExternalInput":
            if name != partition_name:
                in_names.append(name)
        elif alloc.kind == "ExternalOutput":
            out_names.append(name)
            np_dt = mybir.dt.np(alloc.dtype)
            out_avals.append(
                jax.core.ShapedArray(tuple(alloc.tensor_shape), np_dt)
            )
            zero_outs.append(np.zeros(tuple(alloc.tensor_shape), np_dt))
    n_params = len(in_names)
    all_in_names = in_names + out_names
    if partition_name is not None:
        all_in_names = all_in_names + [partition_name]

    def _body(*args):
        operands = list(args)
        if partition_name is not None:
            operands.append(partition_id_tensor())
        outs = _bass_exec_p.bind(
            *operands,
            out_avals=tuple(out_avals),
            in_names=tuple(all_in_names),
            out_names=tuple(out_names),
            lowering_input_output_aliases=(),
            sim_require_finite=True,
            sim_require_nnan=True,
            nc=nc,
        )
        return tuple(outs)

    devices = jax.devices()[:N_CORES]
    mesh = Mesh(np.asarray(devices), ("core",))
    nin = n_params + len(out_names)
    sharded = jax.jit(
        shard_map(
            _body,
            mesh=mesh,
            in_specs=(PartitionSpec("core"),) * nin,
            out_specs=(PartitionSpec("core"),) * len(out_names),
            check_rep=False,
        ),
        keep_unused=True,
    )
    concat_in = [
        np.concatenate([np.asarray(in_maps[c][nm]) for c in range(N_CORES)], axis=0)
        for nm in in_names
    ]
    concat_zero = [
        np.zeros((N_CORES * z.shape[0], *z.shape[1:]), z.dtype) for z in zero_outs
    ]
    sharding = jax.sharding.NamedSharding(mesh, PartitionSpec("core"))
    dev_in = [jax.device_put(a, sharding) for a in concat_in + concat_zero]

    # warmup (compile + first exec); snapshot the output before any
    # further executions can recycle buffers
    t0 = time.time()
    out_arrs = sharded(*dev_in)
    jax.block_until_ready(out_arrs)
    out_np = np.asarray(out_arrs[0]).copy()
    warm_s = time.time() - t0

    blocked = []
    for _ in range(5):
        t0 = time.perf_counter()
        r = sharded(*dev_in)
        jax.block_until_ready(r)
        blocked.append(time.perf_counter() - t0)

    t0 = time.perf_counter()
    r = None
    for _ in range(iters):
        r = sharded(*dev_in)
    jax.block_until_ready(r)
    pipelined = (time.perf_counter() - t0) / iters

    out = _unshard_out(
        [{"out": out_np[c * GPC : (c + 1) * GPC]} for c in range(N_CORES)]
    )
    timing = {
        "warmup_s": warm_s,
        "blocked_ns": min(blocked) * 1e9,
        "pipelined_ns": pipelined * 1e9,
    }
    return out, timing
